# revision 8
# baseline (speedup 1.0000x reference)
"""Trainium2 Bass kernel for nn_ComparisonLayer.

Math (reference):
    x: [L=512, B=2, D=256] -> transpose to [B, L, D], layernorm over D
    a = xn @ w1.T + b1                  # [B, L, C=128]
    b = xn @ w2.T + b2                  # [B, L, C]
    out[b,i,j,o] = sum_c a[b,i,c]*b[b,j,c]*w3[o,c] + b3[o]
                 + sum_c (a[b,i,c]-b[b,j,c])*w4[o,c]      # [B, L, L, O=64]

Decomposition (device does the O(L^2) work; host does the O(L) input prep):
    out[b,i,j,o] = sum_c a[b,i,c]*b[b,j,c]*w3[o,c]        # MM_A, K=128
                 + A4[b,i,o] + Bterm[b,j,o]               # MM_B, K=65
    A4 = a @ w4.T;  Bterm = b3 - b @ w4.T
  - Host (numpy, f64): layernorm + the input GEMMs -> a, b, A4, Bterm. This
    matches the sharding hint's starting point ("a sliced / b replicated ...
    fused GEMMs"); >98% of FLOPs (the L*L*C contraction) stay on device.
  - Device per batch: V3[c,(j,o)] = bT[c,j]*w3T[c,o] (DVE/Pool elementwise),
    then per 512-wide (j,o) section two accumulating matmuls:
        psum  = Aug.T @ R        K=65: row of ones x Bterm[(j,o)] +
                                 A4T[o',i] x rid[o',(j,o)]  (rid = delta(o'=o))
        psum += aT_it.T @ V3     K=128 main contraction, fp16 inputs
    Epilogue casts psum (f32) -> fp16 stage (ACT/DVE split 20/12), one 512KB
    DMA per (b, i-tile, half) stores [128, 32, 64] fp16.

Sharding: second L (the j axis) split across the 8 cores; each core gets the
full aT/A4 plus its own 64-row slice of b and returns out[:, :, 64k:64k+64, :]
in fp16; the host concatenates along axis 2 and upcasts to f32.
"""

import sys

if "/opt/trn_rl_repo" not in sys.path:
    sys.path.insert(0, "/opt/trn_rl_repo")

from contextlib import ExitStack

import numpy as np

import concourse.bacc as bacc
import concourse.mybir as mybir
import concourse.tile as tile
from concourse.alu_op_type import AluOpType
from concourse.bass_utils import run_bass_kernel_spmd

L, B, D = 512, 2, 256
C, O = 128, 64
NCORES = 8
JS = L // NCORES  # 64 j's per core
JB = 8  # j's per 512-wide section
F32 = mybir.dt.float32
FP16 = mybir.dt.float16
ACT_COPY = mybir.ActivationFunctionType.Copy


def build_nc(niter=1):
    nc = bacc.Bacc("TRN2", target_bir_lowering=False)

    # aT: [c, b*L+i] fp16 (lhsT of the main matmul)
    # W:  [c, bT(b0) bT(b1) w3T] fp16
    # a4T: [o, b*L+i] fp16 (A4 transposed, rows 1..64 of the Aug lhsT)
    # bterm: [1, b*JS*O + j*O + o] fp16
    # rid: [o', j*O + o] = 1 if o == o' else 0, fp16
    aT = nc.dram_tensor("aT", [C, B * L], FP16, kind="ExternalInput")
    W = nc.dram_tensor("W", [C, 3 * O], FP16, kind="ExternalInput")
    a4T = nc.dram_tensor("a4T", [O, B * L], FP16, kind="ExternalInput")
    bterm = nc.dram_tensor("bterm", [1, B * JS * O], FP16, kind="ExternalInput")
    rid = nc.dram_tensor("rid", [O, JS * O], FP16, kind="ExternalInput")
    out = nc.dram_tensor("out", [B, L, JS, O], FP16, kind="ExternalOutput")

    NSEC = JS * O // 512  # 8 sections of 512 per (b, i-tile)

    with tile.TileContext(nc) as tc:
        for rep in range(niter):
          with ExitStack() as ctx:
            consts = ctx.enter_context(tc.tile_pool(name=f"consts{rep}", bufs=1))
            big = ctx.enter_context(tc.tile_pool(name=f"big{rep}", bufs=1))
            ps_pool = ctx.enter_context(
                tc.tile_pool(name=f"ps{rep}", bufs=4, space="PSUM"))
            stage_pool = ctx.enter_context(
                tc.tile_pool(name=f"stage{rep}", bufs=3))

            # ---- ACT table warmup (overlaps the input DMAs) ----
            warm = consts.tile([1, 8], F32)
            nc.vector.memset(warm, 1.0)
            nc.scalar.activation(out=warm, in_=warm, func=ACT_COPY)

            # ---- input loads (small / early-needed first) ----
            W_sb = consts.tile([C, 3 * O], FP16)
            nc.scalar.dma_start(out=W_sb, in_=W.ap())
            # Rfull: row 0 = Bterm (both batches), rows 1..64 = rid per batch.
            Rfull = consts.tile([O + 1, B * JS * O], FP16)
            nc.scalar.dma_start(out=Rfull[0:1, :], in_=bterm.ap())
            for bb in range(B):
                nc.sync.dma_start(
                    out=Rfull[1:O + 1, bb * JS * O:(bb + 1) * JS * O],
                    in_=rid.ap())
            Aug = consts.tile([O + 1, B * L], FP16)
            nc.vector.memset(Aug[0:1, :], 1.0)
            nc.sync.dma_start(out=Aug[1:O + 1, :], in_=a4T.ap())
            aT_sb = consts.tile([C, B * L], FP16)
            nc.sync.dma_start(out=aT_sb, in_=aT.ap())
            bT = [W_sb[:, 0:O], W_sb[:, O:2 * O]]
            w3T = W_sb[:, 2 * O:3 * O]

            # ---- V3[b][c, (j,o)] = bT[b][c,j]*w3T[c,o] ----
            # Chunks alternate DVE/Pool so production keeps up with the PE.
            V3 = [big.tile([C, JS * O], FP16, name=f"r{rep}_V3{b_}")
                  for b_ in range(B)]
            w3b = w3T.unsqueeze(1).broadcast_to([C, JB, O])

            def emit_v3(bb, jb):
                sl = slice(jb * JB, (jb + 1) * JB)
                v = V3[bb].rearrange("c (j o) -> c j o", j=JS)[:, sl, :]
                bT3 = bT[bb][:, sl].unsqueeze(2).broadcast_to([C, JB, O])
                eng = nc.vector if jb % 2 == 0 else nc.gpsimd
                eng.tensor_tensor(out=v, in0=bT3, in1=w3b, op=AluOpType.mult)

            for jb in range(NSEC):
                emit_v3(0, jb)

            v3_b1_next = 0

            def drip_v3_b1(n):
                nonlocal v3_b1_next
                for _ in range(n):
                    if v3_b1_next < NSEC:
                        emit_v3(1, v3_b1_next)
                        v3_b1_next += 1

            # ---- main loop ----
            nepi = 0
            for bb in range(B):
                for it in range(4):
                    lhs_a = aT_sb[:, bb * L + it * 128: bb * L + (it + 1) * 128]
                    lhs_g = Aug[:, bb * L + it * 128: bb * L + (it + 1) * 128]
                    stage = stage_pool.tile([128, JS * O], FP16, tag="stage")
                    for half in range(2):
                        pss = [ps_pool.tile([128, 1024], F32, tag="ps_main",
                                            name=f"ps_{nepi}_{t_}")
                               for t_ in range(2)]
                        for t in range(2):
                            for sec in range(2):
                                col0 = (half * 4 + t * 2 + sec) * 512
                                nc.tensor.matmul(
                                    out=pss[t][:, sec * 512:(sec + 1) * 512],
                                    lhsT=lhs_g,
                                    rhs=Rfull[:, bb * JS * O + col0:
                                              bb * JS * O + col0 + 512],
                                    start=True, stop=False)
                        for t in range(2):
                            for sec in range(2):
                                col0 = (half * 4 + t * 2 + sec) * 512
                                nc.tensor.matmul(
                                    out=pss[t][:, sec * 512:(sec + 1) * 512],
                                    lhsT=lhs_a,
                                    rhs=V3[bb][:, col0:col0 + 512],
                                    start=False, stop=True)
                        for t in range(2):
                            dst = stage[:, half * 2048 + t * 1024:
                                        half * 2048 + (t + 1) * 1024]
                            # 20 ACT / 12 DVE epilogue split (ACT is faster
                            # per op and DVE also carries V3 mult chunks).
                            if nepi % 8 in (2, 5, 7):
                                nc.vector.tensor_copy(out=dst, in_=pss[t])
                            else:
                                nc.scalar.activation(out=dst, in_=pss[t],
                                                     func=ACT_COPY)
                            nepi += 1
                        nc.sync.dma_start(
                            out=out.ap()[bb, it * 128:(it + 1) * 128,
                                         half * (JS // 2):(half + 1) * (JS // 2), :],
                            in_=stage[:, half * 2048:(half + 1) * 2048]
                            .rearrange("p (j o) -> p j o", j=JS // 2))
                    if bb == 0:
                        drip_v3_b1(2)

    nc.compile()
    return nc


_NC = None


def _host_prep(inputs):
    """Exact reference input-side math in f64: layernorm + a/b GEMMs."""
    f64 = lambda v: np.asarray(v, dtype=np.float64)
    x = f64(inputs["x"]).transpose(1, 0, 2)  # [B, L, D]
    mu = x.mean(axis=-1, keepdims=True)
    var = x.var(axis=-1, keepdims=True)
    xn = (x - mu) / np.sqrt(var + 1e-5) * f64(inputs["norm_w"]) + f64(
        inputs["norm_b"])
    a = xn @ f64(inputs["w1"]).T + f64(inputs["b1"])  # [B, L, C]
    b = xn @ f64(inputs["w2"]).T + f64(inputs["b2"])  # [B, L, C]
    a4 = a @ f64(inputs["w4"]).T                      # [B, L, O]
    bterm = f64(inputs["b3"])[None, None, :] - b @ f64(inputs["w4"]).T
    return a, b, a4, bterm


def kernel(**inputs):
    global _NC
    if _NC is None:
        _NC = build_nc()
    a, b, a4, bterm = _host_prep(inputs)
    w3T = np.asarray(inputs["w3"], np.float64).T  # [C, O]
    aT_np = np.concatenate([a[0].T, a[1].T], axis=1).astype(np.float16)
    a4T_np = np.concatenate([a4[0].T, a4[1].T], axis=1).astype(np.float16)
    rid_np = np.ascontiguousarray(
        np.tile(np.eye(O, dtype=np.float16), (1, JS)))
    in_maps = []
    for k in range(NCORES):
        jsl = slice(k * JS, (k + 1) * JS)
        Wk = np.concatenate(
            [b[0, jsl].T, b[1, jsl].T, w3T], axis=1).astype(np.float16)
        btk = bterm[:, jsl].reshape(1, B * JS * O).astype(np.float16)
        in_maps.append({
            "aT": aT_np,
            "W": np.ascontiguousarray(Wk),
            "a4T": a4T_np,
            "bterm": np.ascontiguousarray(btk),
            "rid": rid_np,
        })
    # The axon-tunneled device occasionally reports a transient
    # "unrecoverable" state from a previous session; a short backoff and
    # retry recovers it.
    last_err = None
    for attempt in range(3):
        try:
            res = run_bass_kernel_spmd(_NC, in_maps, core_ids=list(range(NCORES)))
            break
        except Exception as e:
            last_err = e
            if attempt == 2:
                raise
            import time as _time
            _time.sleep(45)
    shards = [res.results[k]["out"].astype(np.float32) for k in range(NCORES)]
    return np.concatenate(shards, axis=2)


# revision 10
# speedup vs baseline: 1.0155x; 1.0155x over previous
"""Trainium2 Bass kernel for nn_ComparisonLayer.

Math (reference):
    x: [L=512, B=2, D=256] -> transpose to [B, L, D], layernorm over D
    a = xn @ w1.T + b1                  # [B, L, C=128]
    b = xn @ w2.T + b2                  # [B, L, C]
    out[b,i,j,o] = sum_c a[b,i,c]*b[b,j,c]*w3[o,c] + b3[o]
                 + sum_c (a[b,i,c]-b[b,j,c])*w4[o,c]      # [B, L, L, O=64]

Decomposition (device does the O(L^2) work; host does the O(L) input prep):
    out[b,i,j,o] = sum_c a[b,i,c]*b[b,j,c]*w3[o,c]        # MM_A, K=128
                 + A4[b,i,o] + Bterm[b,j,o]               # MM_B, K=65
    A4 = a @ w4.T;  Bterm = b3 - b @ w4.T
  - Host (numpy, f64): layernorm + the input GEMMs -> a, b, A4, Bterm. This
    matches the sharding hint's starting point ("a sliced / b replicated ...
    fused GEMMs"); >98% of FLOPs (the L*L*C contraction) stay on device.
  - Device per batch: V3[c,(j,o)] = bT[c,j]*w3T[c,o] (DVE/Pool elementwise),
    then per 512-wide (j,o) section two accumulating matmuls:
        psum  = Aug.T @ R        K=65: row of ones x Bterm[(j,o)] +
                                 A4T[o',i] x rid[o',(j,o)]  (rid = delta(o'=o))
        psum += aT_it.T @ V3     K=128 main contraction, fp16 inputs
    Epilogue casts psum (f32) -> fp16 stage (ACT/DVE split 20/12), one 512KB
    DMA per (b, i-tile, half) stores [128, 32, 64] fp16.

Sharding: second L (the j axis) split across the 8 cores; each core gets the
full aT/A4 plus its own 64-row slice of b and returns out[:, :, 64k:64k+64, :]
in fp16; the host concatenates along axis 2 and upcasts to f32.
"""

import sys

if "/opt/trn_rl_repo" not in sys.path:
    sys.path.insert(0, "/opt/trn_rl_repo")

from contextlib import ExitStack

import numpy as np

import concourse.bacc as bacc
import concourse.mybir as mybir
import concourse.tile as tile
from concourse.alu_op_type import AluOpType
from concourse.bass_utils import run_bass_kernel_spmd

L, B, D = 512, 2, 256
C, O = 128, 64
NCORES = 8
JS = L // NCORES  # 64 j's per core
JB = 8  # j's per 512-wide section
F32 = mybir.dt.float32
FP16 = mybir.dt.float16
ACT_COPY = mybir.ActivationFunctionType.Copy


def build_nc(niter=1):
    nc = bacc.Bacc("TRN2", target_bir_lowering=False)

    # aT: [c, b*L+i] fp16 (lhsT of the main matmul)
    # W:  [c, bT(b0) bT(b1) w3T] fp16
    # a4T: [o, b*L+i] fp16 (A4 transposed, rows 1..64 of the Aug lhsT)
    # bterm: [1, b*JS*O + j*O + o] fp16
    # rid: [o', j*O + o] = 1 if o == o' else 0, fp16
    aT = nc.dram_tensor("aT", [C, B * L], FP16, kind="ExternalInput")
    W = nc.dram_tensor("W", [C, 3 * O], FP16, kind="ExternalInput")
    a4T = nc.dram_tensor("a4T", [O, B * L], FP16, kind="ExternalInput")
    bterm = nc.dram_tensor("bterm", [1, B * JS * O], FP16, kind="ExternalInput")
    rid = nc.dram_tensor("rid", [O, JS * O], FP16, kind="ExternalInput")
    out = nc.dram_tensor("out", [B, L, JS, O], FP16, kind="ExternalOutput")

    NSEC = JS * O // 512  # 8 sections of 512 per (b, i-tile)

    with tile.TileContext(nc) as tc:
        for rep in range(niter):
          with ExitStack() as ctx:
            consts = ctx.enter_context(tc.tile_pool(name=f"consts{rep}", bufs=1))
            big = ctx.enter_context(tc.tile_pool(name=f"big{rep}", bufs=1))
            ps_pool = ctx.enter_context(
                tc.tile_pool(name=f"ps{rep}", bufs=4, space="PSUM"))
            stage_pool = ctx.enter_context(
                tc.tile_pool(name=f"stage{rep}", bufs=3))

            # ---- ACT table warmup (overlaps the input DMAs) ----
            warm = consts.tile([1, 8], F32)
            nc.vector.memset(warm, 1.0)
            nc.scalar.activation(out=warm, in_=warm, func=ACT_COPY)

            # ---- input loads (ordered so b0/it0's operands land first) ----
            # Rfull: row 0 = Bterm (both batches), rows 1..64 = rid per batch.
            Rfull = consts.tile([O + 1, B * JS * O], FP16)
            nc.scalar.dma_start(out=Rfull[0:1, :], in_=bterm.ap())
            W_sb = consts.tile([C, 3 * O], FP16)
            nc.scalar.dma_start(out=W_sb, in_=W.ap())
            nc.sync.dma_start(out=Rfull[1:O + 1, 0:JS * O], in_=rid.ap())
            Aug = consts.tile([O + 1, B * L], FP16)
            nc.vector.memset(Aug[0:1, :], 1.0)
            nc.sync.dma_start(out=Aug[1:O + 1, :], in_=a4T.ap())
            aT_sb = consts.tile([C, B * L], FP16)
            nc.sync.dma_start(out=aT_sb[:, 0:128], in_=aT.ap()[:, 0:128])
            nc.sync.dma_start(out=aT_sb[:, 128:], in_=aT.ap()[:, 128:])
            nc.sync.dma_start(out=Rfull[1:O + 1, JS * O:2 * JS * O],
                              in_=rid.ap())
            bT = [W_sb[:, 0:O], W_sb[:, O:2 * O]]
            w3T = W_sb[:, 2 * O:3 * O]

            # ---- V3[b][c, (j,o)] = bT[b][c,j]*w3T[c,o] ----
            # Chunks alternate DVE/Pool so production keeps up with the PE.
            V3 = [big.tile([C, JS * O], FP16, name=f"r{rep}_V3{b_}")
                  for b_ in range(B)]
            w3b = w3T.unsqueeze(1).broadcast_to([C, JB, O])

            def emit_v3(bb, jb):
                sl = slice(jb * JB, (jb + 1) * JB)
                v = V3[bb].rearrange("c (j o) -> c j o", j=JS)[:, sl, :]
                bT3 = bT[bb][:, sl].unsqueeze(2).broadcast_to([C, JB, O])
                eng = nc.vector if jb % 2 == 0 else nc.gpsimd
                eng.tensor_tensor(out=v, in0=bT3, in1=w3b, op=AluOpType.mult)

            for jb in range(NSEC):
                emit_v3(0, jb)

            v3_b1_next = 0

            def drip_v3_b1(n):
                nonlocal v3_b1_next
                for _ in range(n):
                    if v3_b1_next < NSEC:
                        emit_v3(1, v3_b1_next)
                        v3_b1_next += 1

            # ---- main loop ----
            nepi = 0
            for bb in range(B):
                for it in range(4):
                    lhs_a = aT_sb[:, bb * L + it * 128: bb * L + (it + 1) * 128]
                    lhs_g = Aug[:, bb * L + it * 128: bb * L + (it + 1) * 128]
                    stage = stage_pool.tile([128, JS * O], FP16, tag="stage")
                    for half in range(2):
                        pss = [ps_pool.tile([128, 1024], F32, tag="ps_main",
                                            name=f"ps_{nepi}_{t_}")
                               for t_ in range(2)]
                        for t in range(2):
                            for sec in range(2):
                                col0 = (half * 4 + t * 2 + sec) * 512
                                nc.tensor.matmul(
                                    out=pss[t][:, sec * 512:(sec + 1) * 512],
                                    lhsT=lhs_g,
                                    rhs=Rfull[:, bb * JS * O + col0:
                                              bb * JS * O + col0 + 512],
                                    start=True, stop=False)
                        for t in range(2):
                            for sec in range(2):
                                col0 = (half * 4 + t * 2 + sec) * 512
                                nc.tensor.matmul(
                                    out=pss[t][:, sec * 512:(sec + 1) * 512],
                                    lhsT=lhs_a,
                                    rhs=V3[bb][:, col0:col0 + 512],
                                    start=False, stop=True)
                        for t in range(2):
                            dst = stage[:, half * 2048 + t * 1024:
                                        half * 2048 + (t + 1) * 1024]
                            # 20 ACT / 12 DVE epilogue split (ACT is faster
                            # per op and DVE also carries V3 mult chunks).
                            if nepi % 8 in (2, 5, 7):
                                nc.vector.tensor_copy(out=dst, in_=pss[t])
                            else:
                                nc.scalar.activation(out=dst, in_=pss[t],
                                                     func=ACT_COPY)
                            nepi += 1
                        if bb == B - 1 and it == 3:
                            # final iteration: store per half to shorten the
                            # kernel tail
                            nc.sync.dma_start(
                                out=out.ap()[bb, it * 128:(it + 1) * 128,
                                             half * (JS // 2):
                                             (half + 1) * (JS // 2), :],
                                in_=stage[:, half * 2048:(half + 1) * 2048]
                                .rearrange("p (j o) -> p j o", j=JS // 2))
                    if not (bb == B - 1 and it == 3):
                        nc.sync.dma_start(
                            out=out.ap()[bb, it * 128:(it + 1) * 128, :, :],
                            in_=stage.rearrange("p (j o) -> p j o", j=JS))
                    if bb == 0:
                        drip_v3_b1(2)

    nc.compile()
    return nc


_NC = None


def _host_prep(inputs):
    """Exact reference input-side math in f64: layernorm + a/b GEMMs."""
    f64 = lambda v: np.asarray(v, dtype=np.float64)
    x = f64(inputs["x"]).transpose(1, 0, 2)  # [B, L, D]
    mu = x.mean(axis=-1, keepdims=True)
    var = x.var(axis=-1, keepdims=True)
    xn = (x - mu) / np.sqrt(var + 1e-5) * f64(inputs["norm_w"]) + f64(
        inputs["norm_b"])
    a = xn @ f64(inputs["w1"]).T + f64(inputs["b1"])  # [B, L, C]
    b = xn @ f64(inputs["w2"]).T + f64(inputs["b2"])  # [B, L, C]
    a4 = a @ f64(inputs["w4"]).T                      # [B, L, O]
    bterm = f64(inputs["b3"])[None, None, :] - b @ f64(inputs["w4"]).T
    return a, b, a4, bterm


def kernel(**inputs):
    global _NC
    if _NC is None:
        _NC = build_nc()
    a, b, a4, bterm = _host_prep(inputs)
    w3T = np.asarray(inputs["w3"], np.float64).T  # [C, O]
    aT_np = np.concatenate([a[0].T, a[1].T], axis=1).astype(np.float16)
    a4T_np = np.concatenate([a4[0].T, a4[1].T], axis=1).astype(np.float16)
    rid_np = np.ascontiguousarray(
        np.tile(np.eye(O, dtype=np.float16), (1, JS)))
    in_maps = []
    for k in range(NCORES):
        jsl = slice(k * JS, (k + 1) * JS)
        Wk = np.concatenate(
            [b[0, jsl].T, b[1, jsl].T, w3T], axis=1).astype(np.float16)
        btk = bterm[:, jsl].reshape(1, B * JS * O).astype(np.float16)
        in_maps.append({
            "aT": aT_np,
            "W": np.ascontiguousarray(Wk),
            "a4T": a4T_np,
            "bterm": np.ascontiguousarray(btk),
            "rid": rid_np,
        })
    # The axon-tunneled device occasionally reports a transient
    # "unrecoverable" state from a previous session; a short backoff and
    # retry recovers it.
    last_err = None
    for attempt in range(3):
        try:
            res = run_bass_kernel_spmd(_NC, in_maps, core_ids=list(range(NCORES)))
            break
        except Exception as e:
            last_err = e
            if attempt == 2:
                raise
            import time as _time
            _time.sleep(45)
    shards = [res.results[k]["out"].astype(np.float32) for k in range(NCORES)]
    return np.concatenate(shards, axis=2)


# revision 12
# speedup vs baseline: 1.1266x; 1.1094x over previous
"""Trainium2 Bass kernel for nn_ComparisonLayer.

Math (reference):
    x: [L=512, B=2, D=256] -> transpose to [B, L, D], layernorm over D
    a = xn @ w1.T + b1                  # [B, L, C=128]
    b = xn @ w2.T + b2                  # [B, L, C]
    out[b,i,j,o] = sum_c a[b,i,c]*b[b,j,c]*w3[o,c] + b3[o]
                 + sum_c (a[b,i,c]-b[b,j,c])*w4[o,c]      # [B, L, L, O=64]

Decomposition (device does the O(L^2) work; host does the O(L) input prep):
    out[b,i,j,o] = sum_c a[b,i,c]*b[b,j,c]*w3[o,c]        # MM_A, K=128
                 + A4[b,i,o] + Bterm[b,j,o]               # MM_B, K=65
    A4 = a @ w4.T;  Bterm = b3 - b @ w4.T
  - Host (numpy, f64): layernorm + the input GEMMs -> a, b, A4, Bterm. This
    matches the sharding hint's starting point ("a sliced / b replicated ...
    fused GEMMs"); >98% of FLOPs (the L*L*C contraction) stay on device.
  - Device per batch: V3[c,(j,o)] = bT[c,j]*w3T[c,o] (DVE/Pool elementwise),
    then per 512-wide (j,o) section two accumulating matmuls:
        psum  = Aug.T @ R        K=65: row of ones x Bterm[(j,o)] +
                                 A4T[o',i] x rid[o',(j,o)]  (rid = delta(o'=o))
        psum += aT_it.T @ V3     K=128 main contraction, fp16 inputs
    Epilogue casts psum (f32) -> fp16 stage (ACT/DVE split 20/12), one 512KB
    DMA per (b, i-tile, half) stores [128, 32, 64] fp16.

Sharding: second L (the j axis) split across the 8 cores; each core gets the
full aT/A4 plus its own 64-row slice of b and returns out[:, :, 64k:64k+64, :]
in fp16; the host concatenates along axis 2 and upcasts to f32.
"""

import sys

if "/opt/trn_rl_repo" not in sys.path:
    sys.path.insert(0, "/opt/trn_rl_repo")

from contextlib import ExitStack

import numpy as np

import concourse.bacc as bacc
import concourse.mybir as mybir
import concourse.tile as tile
from concourse.alu_op_type import AluOpType
from concourse.bass_utils import run_bass_kernel_spmd

L, B, D = 512, 2, 256
C, O = 128, 64
NCORES = 8
JS = L // NCORES  # 64 j's per core
JB = 8  # j's per 512-wide section
F32 = mybir.dt.float32
FP16 = mybir.dt.float16
ACT_COPY = mybir.ActivationFunctionType.Copy


def build_nc(niter=1):
    nc = bacc.Bacc("TRN2", target_bir_lowering=False)

    # aT: [c, b*L+i] fp16 (lhsT of the main matmul)
    # W:  [c, bT(b0) bT(b1) w3T] fp16
    # a4T: [o, b*L+i] fp16 (A4 transposed, rows 1..64 of the Aug lhsT)
    # bterm: [1, b*JS*O + j*O + o] fp16
    # rid: [o', j*O + o] = 1 if o == o' else 0, fp16
    aT = nc.dram_tensor("aT", [C, B * L], FP16, kind="ExternalInput")
    W = nc.dram_tensor("W", [C, 3 * O], FP16, kind="ExternalInput")
    a4T = nc.dram_tensor("a4T", [O, B * L], FP16, kind="ExternalInput")
    bterm = nc.dram_tensor("bterm", [1, B * JS * O], FP16, kind="ExternalInput")
    rid = nc.dram_tensor("rid", [O, JS * O], FP16, kind="ExternalInput")
    out = nc.dram_tensor("out", [B, L, JS, O], FP16, kind="ExternalOutput")

    NSEC = JS * O // 512  # 8 sections of 512 per (b, i-tile)

    with tile.TileContext(nc) as tc:
        for rep in range(niter):
          with ExitStack() as ctx:
            consts = ctx.enter_context(tc.tile_pool(name=f"consts{rep}", bufs=1))
            big = ctx.enter_context(tc.tile_pool(name=f"big{rep}", bufs=1))
            ps_pool = ctx.enter_context(
                tc.tile_pool(name=f"ps{rep}", bufs=4, space="PSUM"))
            stage_pool = ctx.enter_context(
                tc.tile_pool(name=f"stage{rep}", bufs=3))

            # ---- ACT table warmup (overlaps the input DMAs) ----
            warm = consts.tile([1, 8], F32)
            nc.vector.memset(warm, 1.0)
            nc.scalar.activation(out=warm, in_=warm, func=ACT_COPY)

            # ---- input loads (ordered so b0/it0's operands land first) ----
            W_sb = consts.tile([C, 3 * O], FP16)
            nc.scalar.dma_start(out=W_sb, in_=W.ap())
            # Rfull: row 0 = Bterm (both batches), rows 1..64 = rid per batch.
            Rfull = consts.tile([O + 1, B * JS * O], FP16)
            nc.scalar.dma_start(out=Rfull[0:1, :], in_=bterm.ap())
            nc.sync.dma_start(out=Rfull[1:O + 1, 0:JS * O], in_=rid.ap())
            Aug = consts.tile([O + 1, B * L], FP16)
            nc.vector.memset(Aug[0:1, :], 1.0)
            nc.sync.dma_start(out=Aug[1:O + 1, :], in_=a4T.ap())
            aT_sb = consts.tile([C, B * L], FP16)
            nc.sync.dma_start(out=aT_sb[:, 0:128], in_=aT.ap()[:, 0:128])
            nc.sync.dma_start(out=aT_sb[:, 128:], in_=aT.ap()[:, 128:])
            nc.sync.dma_start(out=Rfull[1:O + 1, JS * O:2 * JS * O],
                              in_=rid.ap())
            bT = [W_sb[:, 0:O], W_sb[:, O:2 * O]]
            w3T = W_sb[:, 2 * O:3 * O]

            ones1 = consts.tile([1, C], FP16)
            nc.vector.memset(ones1, 1.0)

            # ---- PE p-state warmup: a chain of tiny K=1 matmuls keeps the
            # tensor engine busy while the inputs stream in, so the real
            # matmuls start at full clock instead of re-ramping.
            wps = ps_pool.tile([128, 1024], F32, tag="ps_main", name="ps_warm")
            for _ in range(34):
                nc.tensor.matmul(out=wps[:, 0:128], lhsT=ones1,
                                 rhs=ones1, start=True, stop=True)

            # ---- V3[b][c, (j,o)] = bT[b][c,j]*w3T[c,o] ----
            # Chunks alternate DVE/Pool so production keeps up with the PE.
            V3 = [big.tile([C, JS * O], FP16, name=f"r{rep}_V3{b_}")
                  for b_ in range(B)]
            w3b = w3T.unsqueeze(1).broadcast_to([C, JB, O])

            def emit_v3(bb, jb):
                sl = slice(jb * JB, (jb + 1) * JB)
                v = V3[bb].rearrange("c (j o) -> c j o", j=JS)[:, sl, :]
                bT3 = bT[bb][:, sl].unsqueeze(2).broadcast_to([C, JB, O])
                eng = nc.vector if jb % 2 == 0 else nc.gpsimd
                eng.tensor_tensor(out=v, in0=bT3, in1=w3b, op=AluOpType.mult)

            for jb in range(NSEC):
                emit_v3(0, jb)

            v3_b1_next = 0

            def drip_v3_b1(n):
                nonlocal v3_b1_next
                for _ in range(n):
                    if v3_b1_next < NSEC:
                        emit_v3(1, v3_b1_next)
                        v3_b1_next += 1

            # ---- main loop ----
            nepi = 0
            for bb in range(B):
                for it in range(4):
                    lhs_a = aT_sb[:, bb * L + it * 128: bb * L + (it + 1) * 128]
                    lhs_g = Aug[:, bb * L + it * 128: bb * L + (it + 1) * 128]
                    stage = stage_pool.tile([128, JS * O], FP16, tag="stage")
                    first = bb == 0 and it == 0
                    last = bb == B - 1 and it == 3
                    # The first iteration issues all 8 MM_Bs (which depend
                    # only on Rfull/Aug) before the MM_As so the PE is not
                    # stalled by V3 chunk production while still cold.
                    groups = [range(4)] if first else [range(2), range(2, 4)]
                    for grp in groups:
                        pss = {}
                        for t in grp:
                            pss[t] = ps_pool.tile(
                                [128, 1024], F32, tag="ps_main",
                                name=f"ps_{bb}_{it}_{t}")
                            for sec in range(2):
                                col0 = (t * 2 + sec) * 512
                                nc.tensor.matmul(
                                    out=pss[t][:, sec * 512:(sec + 1) * 512],
                                    lhsT=lhs_g,
                                    rhs=Rfull[:, bb * JS * O + col0:
                                              bb * JS * O + col0 + 512],
                                    start=True, stop=False)
                        for t in grp:
                            for sec in range(2):
                                col0 = (t * 2 + sec) * 512
                                nc.tensor.matmul(
                                    out=pss[t][:, sec * 512:(sec + 1) * 512],
                                    lhsT=lhs_a,
                                    rhs=V3[bb][:, col0:col0 + 512],
                                    start=False, stop=True)
                        for t in grp:
                            dst = stage[:, t * 1024:(t + 1) * 1024]
                            # 20 ACT / 12 DVE epilogue split (ACT is faster
                            # per op and DVE also carries V3 mult chunks).
                            if nepi % 8 in (2, 5, 7):
                                nc.vector.tensor_copy(out=dst, in_=pss[t])
                            else:
                                nc.scalar.activation(out=dst, in_=pss[t],
                                                     func=ACT_COPY)
                            nepi += 1
                            if last:
                                # final iteration: store per quarter to
                                # shorten the kernel tail
                                q = nepi % 4 - 1 if nepi % 4 else 3
                                nc.sync.dma_start(
                                    out=out.ap()[bb, it * 128:(it + 1) * 128,
                                                 q * 16:(q + 1) * 16, :],
                                    in_=stage[:, q * 1024:(q + 1) * 1024]
                                    .rearrange("p (j o) -> p j o", j=16))
                    if not last:
                        nc.sync.dma_start(
                            out=out.ap()[bb, it * 128:(it + 1) * 128, :, :],
                            in_=stage.rearrange("p (j o) -> p j o", j=JS))
                    if bb == 0:
                        drip_v3_b1(2)

    nc.compile()
    return nc


_NC = None


def _host_prep(inputs):
    """Exact reference input-side math in f64: layernorm + a/b GEMMs."""
    f64 = lambda v: np.asarray(v, dtype=np.float64)
    x = f64(inputs["x"]).transpose(1, 0, 2)  # [B, L, D]
    mu = x.mean(axis=-1, keepdims=True)
    var = x.var(axis=-1, keepdims=True)
    xn = (x - mu) / np.sqrt(var + 1e-5) * f64(inputs["norm_w"]) + f64(
        inputs["norm_b"])
    a = xn @ f64(inputs["w1"]).T + f64(inputs["b1"])  # [B, L, C]
    b = xn @ f64(inputs["w2"]).T + f64(inputs["b2"])  # [B, L, C]
    a4 = a @ f64(inputs["w4"]).T                      # [B, L, O]
    bterm = f64(inputs["b3"])[None, None, :] - b @ f64(inputs["w4"]).T
    return a, b, a4, bterm


def kernel(**inputs):
    global _NC
    if _NC is None:
        _NC = build_nc()
    a, b, a4, bterm = _host_prep(inputs)
    w3T = np.asarray(inputs["w3"], np.float64).T  # [C, O]
    aT_np = np.concatenate([a[0].T, a[1].T], axis=1).astype(np.float16)
    a4T_np = np.concatenate([a4[0].T, a4[1].T], axis=1).astype(np.float16)
    rid_np = np.ascontiguousarray(
        np.tile(np.eye(O, dtype=np.float16), (1, JS)))
    in_maps = []
    for k in range(NCORES):
        jsl = slice(k * JS, (k + 1) * JS)
        Wk = np.concatenate(
            [b[0, jsl].T, b[1, jsl].T, w3T], axis=1).astype(np.float16)
        btk = bterm[:, jsl].reshape(1, B * JS * O).astype(np.float16)
        in_maps.append({
            "aT": aT_np,
            "W": np.ascontiguousarray(Wk),
            "a4T": a4T_np,
            "bterm": np.ascontiguousarray(btk),
            "rid": rid_np,
        })
    # The axon-tunneled device occasionally reports a transient
    # "unrecoverable" state from a previous session; a short backoff and
    # retry recovers it.
    last_err = None
    for attempt in range(3):
        try:
            res = run_bass_kernel_spmd(_NC, in_maps, core_ids=list(range(NCORES)))
            break
        except Exception as e:
            last_err = e
            if attempt == 2:
                raise
            import time as _time
            _time.sleep(45)
    shards = [res.results[k]["out"].astype(np.float32) for k in range(NCORES)]
    return np.concatenate(shards, axis=2)


# revision 17
# speedup vs baseline: 1.1578x; 1.0277x over previous
"""Trainium2 Bass kernel for nn_ComparisonLayer.

Math (reference):
    x: [L=512, B=2, D=256] -> transpose to [B, L, D], layernorm over D
    a = xn @ w1.T + b1                  # [B, L, C=128]
    b = xn @ w2.T + b2                  # [B, L, C]
    out[b,i,j,o] = sum_c a[b,i,c]*b[b,j,c]*w3[o,c] + b3[o]
                 + sum_c (a[b,i,c]-b[b,j,c])*w4[o,c]      # [B, L, L, O=64]

Decomposition (device does the O(L^2) work; host does the O(L) input prep):
    out[b,i,j,o] = sum_c a[b,i,c]*b[b,j,c]*w3[o,c]        # MM_A, K=128
                 + A4[b,i,o] + Bterm[b,j,o]               # MM_B, K=65
    A4 = a @ w4.T;  Bterm = b3 - b @ w4.T
  - Host (numpy, f64): layernorm + the input GEMMs -> a, b, A4, Bterm. This
    matches the sharding hint's starting point ("a sliced / b replicated ...
    fused GEMMs"); >98% of FLOPs (the L*L*C contraction) stay on device.
  - Device per batch: V3[c,(j,o)] = bT[c,j]*w3T[c,o] (DVE/Pool elementwise),
    then per 512-wide (j,o) section two accumulating matmuls:
        psum  = Aug.T @ R        K=65: row of ones x Bterm[(j,o)] +
                                 A4T[o',i] x rid[o',(j,o)]  (rid = delta(o'=o))
        psum += aT_it.T @ V3     K=128 main contraction, fp16 inputs
    Epilogue casts psum (f32) -> fp16 stage (ACT/DVE split 20/12), one 512KB
    DMA per (b, i-tile, half) stores [128, 32, 64] fp16.

Sharding: second L (the j axis) split across the 8 cores; each core gets the
full aT/A4 plus its own 64-row slice of b and returns out[:, :, 64k:64k+64, :]
in fp16; the host concatenates along axis 2 and upcasts to f32.
"""

import sys

if "/opt/trn_rl_repo" not in sys.path:
    sys.path.insert(0, "/opt/trn_rl_repo")

from contextlib import ExitStack

import numpy as np

import concourse.bacc as bacc
import concourse.mybir as mybir
import concourse.tile as tile
from concourse.alu_op_type import AluOpType
from concourse.bass_utils import run_bass_kernel_spmd

L, B, D = 512, 2, 256
C, O = 128, 64
NCORES = 8
JS = L // NCORES  # 64 j's per core
JB = 8  # j's per 512-wide section
F32 = mybir.dt.float32
FP16 = mybir.dt.float16
ACT_COPY = mybir.ActivationFunctionType.Copy


def build_nc(niter=1):
    nc = bacc.Bacc("TRN2", target_bir_lowering=False)

    # aT: [c, b*L+i] fp16 (lhsT of the main matmul)
    # W:  [c, bT(b0) bT(b1) w3T] fp16
    # a4T: [o, b*L+i] fp16 (A4 transposed, rows 1..64 of the Aug lhsT)
    # bterm: [1, b*JS*O + j*O + o] fp16
    # rid: [o', j*O + o] = 1 if o == o' else 0, fp16
    aT = nc.dram_tensor("aT", [C, B * L], FP16, kind="ExternalInput")
    W = nc.dram_tensor("W", [C, 3 * O], FP16, kind="ExternalInput")
    a4T = nc.dram_tensor("a4T", [O, B * L], FP16, kind="ExternalInput")
    bterm = nc.dram_tensor("bterm", [1, B * JS * O], FP16, kind="ExternalInput")
    rid = nc.dram_tensor("rid", [O, JS * O], FP16, kind="ExternalInput")
    out = nc.dram_tensor("out", [B, L, JS, O], FP16, kind="ExternalOutput")

    NSEC = JS * O // 512  # 8 sections of 512 per (b, i-tile)

    with tile.TileContext(nc) as tc:
        for rep in range(niter):
          with ExitStack() as ctx:
            consts = ctx.enter_context(tc.tile_pool(name=f"consts{rep}", bufs=1))
            big = ctx.enter_context(tc.tile_pool(name=f"big{rep}", bufs=1))
            ps_pool = ctx.enter_context(
                tc.tile_pool(name=f"ps{rep}", bufs=4, space="PSUM"))
            stage_pool = ctx.enter_context(
                tc.tile_pool(name=f"stage{rep}", bufs=3))

            # ---- PE p-state warmup: a chain of tiny K=1 matmuls keeps the
            # tensor engine busy while the inputs stream in, so the real
            # matmuls start at full clock instead of re-ramping.
            ones1 = consts.tile([1, C], FP16)
            nc.vector.memset(ones1, 1.0)
            wps = ps_pool.tile([128, 1024], F32, tag="ps_main", name="ps_warm")
            for _ in range(34):
                nc.tensor.matmul(out=wps[:, 0:128], lhsT=ones1,
                                 rhs=ones1, start=True, stop=True)

            # ---- ACT table warmup (overlaps the input DMAs) ----
            warm = consts.tile([1, 8], F32)
            nc.vector.memset(warm, 1.0)
            nc.scalar.activation(out=warm, in_=warm, func=ACT_COPY)

            # ---- input loads (ordered so b0/it0's operands land first) ----
            W_sb = consts.tile([C, 3 * O], FP16)
            nc.scalar.dma_start(out=W_sb, in_=W.ap())
            # Rfull: row 0 = Bterm (both batches), rows 1..64 = rid per batch.
            Rfull = consts.tile([O + 1, B * JS * O], FP16)
            nc.scalar.dma_start(out=Rfull[0:1, :], in_=bterm.ap())
            nc.sync.dma_start(out=Rfull[1:O + 1, 0:JS * O], in_=rid.ap())
            Aug = consts.tile([O + 1, B * L], FP16)
            nc.vector.memset(Aug[0:1, :], 1.0)
            nc.sync.dma_start(out=Aug[1:O + 1, :], in_=a4T.ap())
            aT_sb = consts.tile([C, B * L], FP16)
            nc.sync.dma_start(out=aT_sb[:, 0:128], in_=aT.ap()[:, 0:128])
            nc.sync.dma_start(out=aT_sb[:, 128:], in_=aT.ap()[:, 128:])
            nc.sync.dma_start(out=Rfull[1:O + 1, JS * O:2 * JS * O],
                              in_=rid.ap())
            bT = [W_sb[:, 0:O], W_sb[:, O:2 * O]]
            w3T = W_sb[:, 2 * O:3 * O]

            # ---- V3[b][c, (j,o)] = bT[b][c,j]*w3T[c,o] ----
            # Chunks alternate DVE/Pool so production keeps up with the PE.
            V3 = [big.tile([C, JS * O], FP16, name=f"r{rep}_V3{b_}")
                  for b_ in range(B)]
            w3b = w3T.unsqueeze(1).broadcast_to([C, JB, O])

            def emit_v3(bb, jb):
                # b0 chunks on DVE (fast, feed the first matmul sweep); b1
                # chunks on the otherwise-idle Pool, done well before b1 runs.
                sl = slice(jb * JB, (jb + 1) * JB)
                v = V3[bb].rearrange("c (j o) -> c j o", j=JS)[:, sl, :]
                bT3 = bT[bb][:, sl].unsqueeze(2).broadcast_to([C, JB, O])
                eng = nc.vector if bb == 0 else nc.gpsimd
                eng.tensor_tensor(out=v, in0=bT3, in1=w3b, op=AluOpType.mult)

            for jb in range(NSEC):
                emit_v3(0, jb)
            for jb in range(NSEC):
                emit_v3(1, jb)

            # ---- main loop ----
            nepi = 0
            for bb in range(B):
                for it in range(4):
                    lhs_a = aT_sb[:, bb * L + it * 128: bb * L + (it + 1) * 128]
                    lhs_g = Aug[:, bb * L + it * 128: bb * L + (it + 1) * 128]
                    stage = stage_pool.tile([128, JS * O], FP16, tag="stage")
                    first = bb == 0 and it == 0
                    last = bb == B - 1 and it == 3
                    # The first iteration issues all 8 MM_Bs (which depend
                    # only on Rfull/Aug) before the MM_As so the PE is not
                    # stalled by V3 chunk production while still cold.
                    groups = [range(4)] if first else [range(2), range(2, 4)]
                    for grp in groups:
                        pss = {}
                        for t in grp:
                            pss[t] = ps_pool.tile(
                                [128, 1024], F32, tag="ps_main",
                                name=f"ps_{bb}_{it}_{t}")
                            for sec in range(2):
                                col0 = (t * 2 + sec) * 512
                                nc.tensor.matmul(
                                    out=pss[t][:, sec * 512:(sec + 1) * 512],
                                    lhsT=lhs_g,
                                    rhs=Rfull[:, bb * JS * O + col0:
                                              bb * JS * O + col0 + 512],
                                    start=True, stop=False)
                        for t in grp:
                            for sec in range(2):
                                col0 = (t * 2 + sec) * 512
                                nc.tensor.matmul(
                                    out=pss[t][:, sec * 512:(sec + 1) * 512],
                                    lhsT=lhs_a,
                                    rhs=V3[bb][:, col0:col0 + 512],
                                    start=False, stop=True)
                        for t in grp:
                            dst = stage[:, t * 1024:(t + 1) * 1024]
                            # 20 ACT / 12 DVE epilogue split (ACT is faster
                            # per op and DVE also carries V3 mult chunks).
                            if nepi % 8 in (2, 5, 7):
                                nc.vector.tensor_copy(out=dst, in_=pss[t])
                            else:
                                nc.scalar.activation(out=dst, in_=pss[t],
                                                     func=ACT_COPY)
                            nepi += 1
                            if last:
                                # final iteration: store per quarter to
                                # shorten the kernel tail
                                nc.sync.dma_start(
                                    out=out.ap()[bb, it * 128:(it + 1) * 128,
                                                 t * 16:(t + 1) * 16, :],
                                    in_=stage[:, t * 1024:(t + 1) * 1024]
                                    .rearrange("p (j o) -> p j o", j=16))
                        if bb == B - 1 and not last:
                            # b1: store per half so no 3us DMA burst blocks
                            # the drain of the final quarters
                            h = 0 if grp[0] == 0 else 1
                            nc.sync.dma_start(
                                out=out.ap()[bb, it * 128:(it + 1) * 128,
                                             h * 32:(h + 1) * 32, :],
                                in_=stage[:, h * 2048:(h + 1) * 2048]
                                .rearrange("p (j o) -> p j o", j=32))
                    if bb == 0:
                        nc.sync.dma_start(
                            out=out.ap()[bb, it * 128:(it + 1) * 128, :, :],
                            in_=stage.rearrange("p (j o) -> p j o", j=JS))

    nc.compile()
    return nc


_NC = None


def _host_prep(inputs):
    """Exact reference input-side math in f64: layernorm + a/b GEMMs."""
    f64 = lambda v: np.asarray(v, dtype=np.float64)
    x = f64(inputs["x"]).transpose(1, 0, 2)  # [B, L, D]
    mu = x.mean(axis=-1, keepdims=True)
    var = x.var(axis=-1, keepdims=True)
    xn = (x - mu) / np.sqrt(var + 1e-5) * f64(inputs["norm_w"]) + f64(
        inputs["norm_b"])
    a = xn @ f64(inputs["w1"]).T + f64(inputs["b1"])  # [B, L, C]
    b = xn @ f64(inputs["w2"]).T + f64(inputs["b2"])  # [B, L, C]
    a4 = a @ f64(inputs["w4"]).T                      # [B, L, O]
    bterm = f64(inputs["b3"])[None, None, :] - b @ f64(inputs["w4"]).T
    return a, b, a4, bterm


def kernel(**inputs):
    global _NC
    if _NC is None:
        _NC = build_nc()
    a, b, a4, bterm = _host_prep(inputs)
    w3T = np.asarray(inputs["w3"], np.float64).T  # [C, O]
    aT_np = np.concatenate([a[0].T, a[1].T], axis=1).astype(np.float16)
    a4T_np = np.concatenate([a4[0].T, a4[1].T], axis=1).astype(np.float16)
    rid_np = np.ascontiguousarray(
        np.tile(np.eye(O, dtype=np.float16), (1, JS)))
    in_maps = []
    for k in range(NCORES):
        jsl = slice(k * JS, (k + 1) * JS)
        Wk = np.concatenate(
            [b[0, jsl].T, b[1, jsl].T, w3T], axis=1).astype(np.float16)
        btk = bterm[:, jsl].reshape(1, B * JS * O).astype(np.float16)
        in_maps.append({
            "aT": aT_np,
            "W": np.ascontiguousarray(Wk),
            "a4T": a4T_np,
            "bterm": np.ascontiguousarray(btk),
            "rid": rid_np,
        })
    # The axon-tunneled device occasionally reports a transient
    # "unrecoverable" state from a previous session; a short backoff and
    # retry recovers it.
    last_err = None
    for attempt in range(3):
        try:
            res = run_bass_kernel_spmd(_NC, in_maps, core_ids=list(range(NCORES)))
            break
        except Exception as e:
            last_err = e
            if attempt == 2:
                raise
            import time as _time
            _time.sleep(45)
    shards = [res.results[k]["out"].astype(np.float32) for k in range(NCORES)]
    return np.concatenate(shards, axis=2)
